# revision 1
# baseline (speedup 1.0000x reference)
"""Trainium2 Bass kernel for nn_LongRangeDW (dense_cnn).

The module is entirely linear in x:
  s = nnstacking(x)                        (5 shifted copies, clipped to window)
  y = dw1(s) + dw2(s) + dw3(s)             (depthwise 1x1 + 3x3 d8 + 3x3 d12)
  out = pw(y) + x                          (pointwise 5C->C + residual)

Folding the depthwise taps into the pointwise gives, per nnstacking group g
with shift sigma_g and tap tau:
  out[o, p] = sum_{g,t} (W4_g diag(k_{g,t}))[o,:] @ xe[:, p + tau_t + sigma_g]
              + beff[o] + x[o, p]
with xe = zero-extended x: 85 distinct offsets -> 128x128 bf16 matrices applied
to shifted views of a zero-padded SBUF-resident image, accumulated in PSUM by
the tensor engine (1 column/cycle).

Two groups are NOT run as matmuls: their depthwise outputs y_g are built as
per-channel-scalar tap FMAs on the Scalar (multiply) + Vector (accumulate)
engines, which are otherwise idle, followed by a single pointwise matmul term
per group. This removes 17 matmul terms per offloaded group at the cost of one.

Boundary exactness: composing clipped shifts with zero-padded convs is NOT the
padded composite. Where a depthwise tap lands exactly 1 px outside the window
and sigma_g pulls it back in, the composite wrongly reads x. The mismatch
lives on 8 one-pixel strips (output rows/cols {7,11,116,120}) reading x's 4
border lines -> 24 small correction matmuls folded in during evacuation.
The residual + bias are applied in exact fp32 during PSUM evacuation.

Data parallel: batch B=8 -> one image per NeuronCore.
"""

import sys

import numpy as np

sys.path.insert(0, "/opt/trn_rl_repo")

B, C, H, W = 8, 128, 128, 128
PAD = 14            # max |offset| = 13, rounded even for DVE 4B alignment
HP = H + 2 * PAD
WP = W + 2 * PAD
N_CORES = 8
SB_ROWS = 8         # output rows per super-block (psum tile = 2 banks)
N_SB = H // SB_ROWS
SUB_ROWS = 4        # rows per matmul (out free dim 512 = one PSUM bank)

SHIFTS = [(1, 0), (-1, 0), (0, 1), (0, -1), (0, 0)]  # nnstacking groups

# (group, taps offloaded to Scalar+Vector engines); tune counts to balance
# PE vs DVE/ACT occupancy.
OFFLOAD = [(4, 17), (0, 7)]


# --------------------------------------------------------------------------
# host-side operator folding
# --------------------------------------------------------------------------

def _group_taps(w1, w2, w3, g):
    """All 17 taps of group g as {(di, dj): kvec[C]} (shift folded in)."""
    sy, sx = SHIFTS[g]
    sl = slice(g * C, (g + 1) * C)
    taps = {}

    def add(di, dj, kv):
        v = taps.setdefault((di, dj), np.zeros(C, np.float64))
        v += kv.astype(np.float64)

    add(sy, sx, w1[sl, 0, 0, 0])
    for w, d in ((w2, 8), (w3, 12)):
        for a in range(3):
            for b in range(3):
                add(sy + (a - 1) * d, sx + (b - 1) * d, w[sl, 0, a, b])
    return taps


def _build_terms(w1, w2, w3, w4):
    """Returns (offsets, mats, off_specs) where off_specs is a list per
    OFFLOAD entry: dict(g, tap_offsets, kmat [C, n], w4g [C, C])."""
    w4m = w4[:, :, 0, 0].astype(np.float64)  # [C, 5C]
    offload_n = dict(OFFLOAD)
    mat_terms = {}
    off_specs = []
    for g in range(5):
        taps = _group_taps(w1, w2, w3, g)
        tap_offsets = sorted(taps)
        n_off = offload_n.get(g, 0)
        off, keep = tap_offsets[:n_off], tap_offsets[n_off:]
        if off:
            kmat = np.stack([taps[o] for o in off], axis=1)  # [C, n]
            off_specs.append(dict(
                g=g, tap_offsets=off, kmat=kmat.astype(np.float32),
                w4g=w4m[:, g * C:(g + 1) * C].astype(np.float32)))
        for o in keep:
            M = mat_terms.setdefault(o, np.zeros((C, C), np.float64))
            M += w4m[:, g * C:(g + 1) * C] * taps[o][None, :]
    offsets = sorted(mat_terms)
    mats = np.stack([mat_terms[o] for o in offsets]).astype(np.float32)
    return offsets, mats, off_specs


def _build_corrections(w2, w3, w4):
    """24 strip-correction terms (matrices already NEGATED for accumulation).

    Strips j<4: column strips (out col px, read x col src, row shift ty);
    j>=4: row strips. Each strip has 3 taps."""
    w4m = w4[:, :, 0, 0].astype(np.float64)
    strips, mats = [], []
    specs = [
        ("col", 2, 8), ("col", 2, 12), ("col", 3, 12), ("col", 3, 8),
        ("row", 0, 8), ("row", 0, 12), ("row", 1, 12), ("row", 1, 8),
    ]
    for kind, g, d in specs:
        sy, sx = SHIFTS[g]
        sl = slice(g * C, (g + 1) * C)
        w = w2 if d == 8 else w3
        if kind == "col":
            border = -1 if sx == 1 else W
            fixed_out = border - (-d if sx == 1 else d)
            src = border + sx
            shifts = [-d, 0, d]                     # ty values
            tap_b = 0 if sx == 1 else 2
            kvs = [w[sl, 0, a, tap_b] for a in range(3)]
        else:
            border = -1 if sy == 1 else H
            fixed_out = border - (-d if sy == 1 else d)
            src = border + sy
            shifts = [-d, 0, d]                     # tx values
            tap_a = 0 if sy == 1 else 2
            kvs = [w[sl, 0, tap_a, b] for b in range(3)]
        strips.append(dict(kind=kind, fixed_out=fixed_out, src=src, shifts=shifts))
        for kv in kvs:
            mats.append(-(w4m[:, sl] * kv.astype(np.float64)[None, :]))
    return strips, np.stack(mats).astype(np.float32)


def _build_weights(inputs):
    w1, w2, w3, w4 = inputs["w1"], inputs["w2"], inputs["w3"], inputs["w4"]
    b1, b2, b3, b4 = inputs["b1"], inputs["b2"], inputs["b3"], inputs["b4"]
    offsets, mats, off_specs = _build_terms(w1, w2, w3, w4)
    strips, cmats = _build_corrections(w2, w3, w4)
    pw = np.stack([sp["w4g"] for sp in off_specs])       # [n_off, C, C]
    ident = np.eye(C, dtype=np.float32)[None]
    allm = np.concatenate([mats, cmats, pw, ident], axis=0)  # [*, C(o), C(c)]
    wt = np.ascontiguousarray(allm.transpose(2, 0, 1).reshape(C, -1)).astype(np.float32)
    ks = np.concatenate([sp["kmat"] for sp in off_specs], axis=1)  # [C, ntaps]
    w4m = w4[:, :, 0, 0].astype(np.float64)
    beff = (b4.astype(np.float64)
            + w4m @ (b1 + b2 + b3).astype(np.float64)).astype(np.float32)
    return wt, ks, beff, offsets, off_specs, strips


# --------------------------------------------------------------------------
# device program
# --------------------------------------------------------------------------

_CACHE = {}


def _build_program(offsets, off_specs, strips):
    import concourse.bacc as bacc
    import concourse.mybir as mybir
    import concourse.tile as tile

    nc = bacc.Bacc("TRN2", target_bir_lowering=False)
    f32 = mybir.dt.float32
    bf16 = mybir.dt.bfloat16

    n_terms = len(offsets)
    n_off = len(off_specs)
    n_ks = sum(len(sp["tap_offsets"]) for sp in off_specs)
    CORR_BLK = n_terms
    PW_BLK = n_terms + 24
    ID_BLK = n_terms + 24 + n_off
    n_blk = n_terms + 24 + n_off + 1

    xp_d = nc.dram_tensor("xp", [C, HP * WP], bf16, kind="ExternalInput")
    wt_d = nc.dram_tensor("wt", [C, n_blk * C], bf16, kind="ExternalInput")
    xres_d = nc.dram_tensor("xres", [C, H * W], f32, kind="ExternalInput")
    ks_d = nc.dram_tensor("ks", [C, n_ks], f32, kind="ExternalInput")
    beff_d = nc.dram_tensor("beff", [C, 1], f32, kind="ExternalInput")
    out_d = nc.dram_tensor("out", [C, H * W], f32, kind="ExternalOutput")

    with tile.TileContext(nc) as tc:
        with (
            tc.tile_pool(name="const", bufs=1) as const,
            tc.tile_pool(name="outp", bufs=3) as outp,
            tc.tile_pool(name="tmpp", bufs=4) as tmpp,
            tc.tile_pool(name="yp", bufs=2) as yp,
            tc.tile_pool(name="psum", bufs=3, space="PSUM") as psum_pool,
            tc.tile_pool(name="psumc", bufs=1, space="PSUM") as psumc_pool,
        ):
            xp_sb = const.tile([C, HP * WP], bf16)
            wt_sb = const.tile([C, n_blk * C], bf16)
            xres_sb = const.tile([C, H * W], f32)
            ks_sb = const.tile([C, n_ks], f32)
            beff_sb = const.tile([C, 1], f32)

            # SWDGE (nc.gpsimd) fans >=1MB transfers across all 16 SDMA
            # engines (~340 GB/s); HWDGE runs ~26 GB/s on a single engine.
            # Order: minimum needed for SB0 first.
            WT_C0 = 6 * C
            nc.gpsimd.dma_start(out=wt_sb[:, :WT_C0], in_=wt_d[:, :WT_C0])
            ROWS0 = SB_ROWS + 2 * PAD
            nc.gpsimd.dma_start(out=xp_sb[:, :ROWS0 * WP],
                                in_=xp_d[:, :ROWS0 * WP], )
            WT_C1 = 54 * C
            nc.gpsimd.dma_start(out=wt_sb[:, WT_C0:WT_C1],
                                in_=wt_d[:, WT_C0:WT_C1])
            nc.gpsimd.dma_start(out=wt_sb[:, WT_C1:], in_=wt_d[:, WT_C1:])
            XP_CHUNK_ROWS = 56
            for r0_ in range(ROWS0, HP, XP_CHUNK_ROWS):
                r1_ = min(r0_ + XP_CHUNK_ROWS, HP)
                nc.gpsimd.dma_start(out=xp_sb[:, r0_ * WP:r1_ * WP],
                                    in_=xp_d[:, r0_ * WP:r1_ * WP])
            nc.sync.dma_start(out=beff_sb, in_=beff_d[:, :])
            nc.sync.dma_start(out=ks_sb, in_=ks_d[:, :])
            for q0 in range(0, H * W, H * W // 4):
                q1 = q0 + H * W // 4
                nc.gpsimd.dma_start(out=xres_sb[:, q0:q1], in_=xres_d[:, q0:q1])

            xp3 = xp_sb.rearrange("p (r w) -> p r w", w=WP)

            def wblk(i):
                return wt_sb[:, i * C:(i + 1) * C]

            corr_sb = const.tile([C, 8 * H], bf16)

            def emit_corrections():
                # needs the full xp image -> emitted after SB0's matmuls
                psum_c = psumc_pool.tile([C, 8 * H], f32, name="psum_c")
                for j, st in enumerate(strips):
                    for i, sh in enumerate(st["shifts"]):
                        if st["kind"] == "col":
                            rhs = xp3[:, PAD + sh: PAD + sh + H,
                                      PAD + st["src"]: PAD + st["src"] + 1]
                        else:
                            rhs = xp3[:, PAD + st["src"]: PAD + st["src"] + 1,
                                      PAD + sh: PAD + sh + W]
                        nc.tensor.matmul(psum_c[:, j * H:(j + 1) * H],
                                         wblk(CORR_BLK + 3 * j + i), rhs,
                                         start=(i == 0), stop=(i == 2))
                # ACT, not DVE: DVE is busy with taps; psum_c slot release
                # should not sit behind them
                nc.scalar.copy(corr_sb, psum_c)

            # per-OFFLOAD-group scalar column base in ks
            ks_base = []
            b = 0
            for sp in off_specs:
                ks_base.append(b)
                b += len(sp["tap_offsets"])

            # ---- main loop -------------------------------------------------
            n_sub = SB_ROWS // SUB_ROWS
            Y_ROWS = 2 * SB_ROWS  # tap FMAs at 2-SB granularity (amortize DVE
            pair_ys = None        # per-op overhead); PE consumes halves
            for s in range(N_SB):
                r0 = s * SB_ROWS

                # offloaded groups: per-channel-scalar tap FMAs on the Vector
                # engine (all offsets have even dj -> 4B-aligned bf16 reads)
                if s % 2 == 0:
                    pair_ys = []
                    for oi, sp in enumerate(off_specs):
                        y = yp.tile([C, Y_ROWS * W], bf16, name=f"y{sp['g']}",
                                    tag=f"y{sp['g']}")
                        for t, (dy, dx) in enumerate(sp["tap_offsets"]):
                            xs = xp3[:, PAD + r0 + dy: PAD + r0 + dy + Y_ROWS,
                                     PAD + dx: PAD + dx + W]
                            kcol = ks_sb[:, ks_base[oi] + t: ks_base[oi] + t + 1]
                            if t == 0:
                                nc.vector.tensor_scalar_mul(y, xs, kcol)
                            else:
                                nc.vector.scalar_tensor_tensor(
                                    y, xs, kcol, y,
                                    mybir.AluOpType.mult, mybir.AluOpType.add)
                        pair_ys.append(y.rearrange("p (r w) -> p r w", w=W))
                half = (s % 2) * SB_ROWS
                ys = [y3[:, half:half + SB_ROWS, :] for y3 in pair_ys]

                psum = psum_pool.tile([C, SB_ROWS * W], f32, tag="acc")
                for t, (di, dj) in enumerate(offsets):
                    for u in range(n_sub):
                        a0 = PAD + r0 + u * SUB_ROWS + di
                        rhs = xp3[:, a0: a0 + SUB_ROWS, PAD + dj: PAD + dj + W]
                        nc.tensor.matmul(
                            psum[:, u * SUB_ROWS * W:(u + 1) * SUB_ROWS * W],
                            wblk(t), rhs,
                            start=(t == 0), stop=False)
                for oi in range(n_off):
                    for u in range(n_sub):
                        nc.tensor.matmul(
                            psum[:, u * SUB_ROWS * W:(u + 1) * SUB_ROWS * W],
                            wblk(PW_BLK + oi),
                            ys[oi][:, u * SUB_ROWS:(u + 1) * SUB_ROWS, :],
                            start=False, stop=False)

                if s == 0:
                    emit_corrections()

                # fold strip corrections into PSUM on the PE: identity-weight
                # matmuls add corr_sb rows into strided psum positions
                psum3 = psum.rearrange("p (r w) -> p r w", w=W)
                strip_mms = []
                for j, st in enumerate(strips):
                    if st["kind"] == "col":
                        dst = psum3[:, 0:SB_ROWS,
                                    st["fixed_out"]:st["fixed_out"] + 1]
                        src = corr_sb[:, j * H + r0: j * H + r0 + SB_ROWS]
                        strip_mms.append((dst, src))
                    elif r0 <= st["fixed_out"] < r0 + SB_ROWS:
                        lr = st["fixed_out"] - r0
                        strip_mms.append((psum3[:, lr:lr + 1, :],
                                          corr_sb[:, j * H: j * H + W]))
                for i, (dst, src) in enumerate(strip_mms):
                    nc.tensor.matmul(dst, wblk(ID_BLK), src,
                                     start=False, stop=(i == len(strip_mms) - 1))

                out_sb = outp.tile([C, SB_ROWS * W], f32)
                nc.scalar.activation(out_sb, psum,
                                     mybir.ActivationFunctionType.Identity,
                                     bias=beff_sb[:, 0:1])
                nc.vector.tensor_add(
                    out_sb, out_sb,
                    xres_sb[:, r0 * W:(r0 + SB_ROWS) * W])
                nc.gpsimd.dma_start(out=out_d[:, r0 * W:(r0 + SB_ROWS) * W],
                                    in_=out_sb)
    nc.finalize()
    return nc


def _make_in_maps(inputs):
    x = np.ascontiguousarray(inputs["x"], dtype=np.float32)
    wt, ks, beff, offsets, off_specs, strips = _build_weights(inputs)
    if "nc" not in _CACHE:
        _CACHE["nc"] = _build_program(offsets, off_specs, strips)

    import ml_dtypes
    bf = ml_dtypes.bfloat16
    xpad = np.zeros((B, C, HP, WP), bf)
    xpad[:, :, PAD:PAD + H, PAD:PAD + W] = x.astype(bf)
    beff_col = np.ascontiguousarray(beff.reshape(C, 1))
    wt_bf16 = wt.astype(bf)
    ksc = np.ascontiguousarray(ks)
    return [
        {
            "xp": np.ascontiguousarray(xpad[b].reshape(C, HP * WP)),
            "wt": wt_bf16,
            "xres": np.ascontiguousarray(x[b].reshape(C, H * W)),
            "ks": ksc,
            "beff": beff_col,
        }
        for b in range(B)
    ]


def kernel(**inputs):
    in_maps = _make_in_maps(inputs)
    from concourse.bass_utils import run_bass_kernel_spmd
    res = run_bass_kernel_spmd(_CACHE["nc"], in_maps, core_ids=list(range(N_CORES)))
    out = np.stack([res.results[b]["out"].reshape(C, H, W) for b in range(B)])
    return out.astype(np.float32)



# revision 3
# speedup vs baseline: 1.8242x; 1.8242x over previous
"""Trainium2 Bass kernel for nn_LongRangeDW (dense_cnn).

The module is linear in x:
  s = nnstacking(x)                        (5 shifted copies, clipped)
  y = dw1(s) + dw2(s) + dw3(s)             (depthwise 1x1 + 3x3 d8 + 3x3 d12)
  out = pw(y) + x                          (pointwise 5C->C + residual)

Folding depthwise taps into the pointwise yields 85 terms
  out[o, p] = sum_t M_t[o, :] @ xe[:, p + off_t]  (+ bias + residual)
with xe the zero-extended image. Terms are packed TWO AT A TIME into fp8
DoubleRow matmuls: the PE computes lhsT[:,0].T @ rhs[:,0] + lhsT[:,1].T @
rhs[:,1] in one pass at 2 columns-of-contraction per cycle, so a pair of
128x128 terms costs the same as one bf16 term. The paired rhs is a custom
4D access pattern [part, 2(pair-stride), rows, cols] over one fp8 SBUF
image; pairing is unrestricted because the pair stride is arbitrary.
fp8 weights are pre-scaled by 2^7 (entries ~0.002 would be e4m3
subnormals); the scale is divided out during PSUM evacuation (ACT scale).

A few taps of the center nnstacking group are offloaded to the Vector
engine (4x-mode tensor_scalar products + 2x-mode tensor_tensor adds over
full padded-width rows) and re-enter the PE as one bf16 pointwise matmul.

Boundary exactness: composing clipped shifts with zero-padded convs is NOT
the padded composite. Where a depthwise tap lands 1 px outside the window
and the nnstacking shift pulls it back in, the composite wrongly reads x.
The mismatch lives on 8 one-pixel strips (output rows/cols {7,11,116,120})
reading x's 4 border lines -> 24 small correction matmuls folded in via
identity-weight matmuls during accumulation. Bias + residual are applied
in exact fp32 during/after PSUM evacuation.

Data parallel: batch B=8 -> one image per NeuronCore.
"""

import sys

import numpy as np

sys.path.insert(0, "/opt/trn_rl_repo")

B, C, H, W = 8, 128, 128, 128
PAD = 14            # max |offset| = 13, rounded even
HP = H + 2 * PAD
WP = W + 2 * PAD
N_CORES = 8
SB_ROWS = 8         # output rows per super-block (psum tile = 2 banks)
N_SB = H // SB_ROWS
SUB_ROWS = 4        # rows per matmul (out free dim 512 = one PSUM bank)
Y_ROWS = 16         # tap-FMA block rows (2 super-blocks)

SHIFTS = [(1, 0), (-1, 0), (0, 1), (0, -1), (0, 0)]  # nnstacking groups

OFF_G = 4           # group whose taps go to the Vector engine
OFF_N = 13          # how many of its 17 taps are offloaded
WSCALE = 128.0      # fp8 weight pre-scale (2^7), divided out at evacuation


# --------------------------------------------------------------------------
# host-side operator folding
# --------------------------------------------------------------------------

def _group_taps(w1, w2, w3, g):
    """All 17 taps of group g as {(di, dj): kvec[C]} (shift folded in)."""
    sy, sx = SHIFTS[g]
    sl = slice(g * C, (g + 1) * C)
    taps = {}

    def add(di, dj, kv):
        v = taps.setdefault((di, dj), np.zeros(C, np.float64))
        v += kv.astype(np.float64)

    add(sy, sx, w1[sl, 0, 0, 0])
    for w, d in ((w2, 8), (w3, 12)):
        for a in range(3):
            for b in range(3):
                add(sy + (a - 1) * d, sx + (b - 1) * d, w[sl, 0, a, b])
    return taps


def _build_terms(w1, w2, w3, w4):
    """Split the 85 (offset, matrix) terms into DR pairs + offloaded taps.

    Returns (pairs, off_spec):
      pairs: list of ((offA, matA), (offB, matB)) with flat(offA) < flat(offB)
      off_spec: dict(tap_offsets [OFF_N], kmat [C, OFF_N], w4g [C, C])
    """
    w4m = w4[:, :, 0, 0].astype(np.float64)
    terms = []          # (offset, matrix) in f64
    off_spec = None
    for g in range(5):
        taps = _group_taps(w1, w2, w3, g)
        tap_offsets = sorted(taps)
        if g == OFF_G:
            off, keep = tap_offsets[:OFF_N], tap_offsets[OFF_N:]
            kmat = np.stack([taps[o] for o in off], axis=1)
            off_spec = dict(
                tap_offsets=off, kmat=kmat.astype(np.float32),
                w4g=w4m[:, g * C:(g + 1) * C].astype(np.float32))
        else:
            keep = tap_offsets
        for o in keep:
            terms.append((o, w4m[:, g * C:(g + 1) * C] * taps[o][None, :]))
    assert len(terms) % 2 == 0, len(terms)
    terms.sort(key=lambda t: t[0])
    n = len(terms) // 2
    pairs = []
    for i in range(n):
        a, b = terms[2 * i], terms[2 * i + 1]
        fa = a[0][0] * WP + a[0][1]
        fb = b[0][0] * WP + b[0][1]
        if fa > fb:
            a, b = b, a
        pairs.append((a, b))
    return pairs, off_spec


def _build_corrections(w2, w3, w4):
    """24 strip-correction terms (matrices already NEGATED).

    Strips j<4: column strips; j>=4: row strips. Each has 3 taps."""
    w4m = w4[:, :, 0, 0].astype(np.float64)
    strips, mats = [], []
    specs = [
        ("col", 2, 8), ("col", 2, 12), ("col", 3, 12), ("col", 3, 8),
        ("row", 0, 8), ("row", 0, 12), ("row", 1, 12), ("row", 1, 8),
    ]
    for kind, g, d in specs:
        sy, sx = SHIFTS[g]
        sl = slice(g * C, (g + 1) * C)
        w = w2 if d == 8 else w3
        if kind == "col":
            border = -1 if sx == 1 else W
            fixed_out = border - (-d if sx == 1 else d)
            src = border + sx
            shifts = [-d, 0, d]
            tap_b = 0 if sx == 1 else 2
            kvs = [w[sl, 0, a, tap_b] for a in range(3)]
        else:
            border = -1 if sy == 1 else H
            fixed_out = border - (-d if sy == 1 else d)
            src = border + sy
            shifts = [-d, 0, d]
            tap_a = 0 if sy == 1 else 2
            kvs = [w[sl, 0, tap_a, b] for b in range(3)]
        strips.append(dict(kind=kind, fixed_out=fixed_out, src=src,
                           shifts=shifts))
        for kv in kvs:
            mats.append(-(w4m[:, sl] * kv.astype(np.float64)[None, :]))
    return strips, np.stack(mats).astype(np.float64)


def _build_weights(inputs):
    import ml_dtypes
    f8 = ml_dtypes.float8_e4m3
    bf = ml_dtypes.bfloat16

    w1, w2, w3, w4 = inputs["w1"], inputs["w2"], inputs["w3"], inputs["w4"]
    b1, b2, b3, b4 = inputs["b1"], inputs["b2"], inputs["b3"], inputs["b4"]
    pairs, off_spec = _build_terms(w1, w2, w3, w4)
    strips, cmats = _build_corrections(w2, w3, w4)

    # fp8 pair weights: wt8[c, i, s, m] = s_w * M_{i,s}[m, c]
    npair = len(pairs)
    wt8 = np.zeros((C, npair, 2, C), np.float64)
    pair_offs = []
    for i, (a, b) in enumerate(pairs):
        pair_offs.append((a[0], b[0]))
        wt8[:, i, 0, :] = a[1].T * WSCALE
        wt8[:, i, 1, :] = b[1].T * WSCALE
    wt8 = np.ascontiguousarray(wt8.reshape(C, npair * 2 * C)).astype(f8)

    # bf16 blocks: 24 corrections, 1 pointwise, 1 identity (all x WSCALE,
    # except identity which adds the already-scaled corr rows)
    blocks = [m.T * WSCALE for m in cmats]            # corr: [C(c), C(o)] -> T
    blocks.append(off_spec["w4g"].astype(np.float64).T * WSCALE)
    blocks.append(np.eye(C))
    wtb = np.stack(blocks, axis=1)                    # [C, nblk, C]
    wtb = np.ascontiguousarray(
        wtb.reshape(C, -1)).astype(bf)

    ks = np.ascontiguousarray(off_spec["kmat"])       # [C, OFF_N] f32
    w4m = w4[:, :, 0, 0].astype(np.float64)
    beff = (b4.astype(np.float64)
            + w4m @ (b1 + b2 + b3).astype(np.float64)).astype(np.float32)
    return wt8, wtb, ks, beff, pair_offs, off_spec["tap_offsets"], strips


# --------------------------------------------------------------------------
# device program
# --------------------------------------------------------------------------

_CACHE = {}


def _build_program(pair_offs, tap_offsets, strips):
    import bass_rust
    import concourse.bacc as bacc
    import concourse.mybir as mybir
    import concourse.tile as tile

    nc = bacc.Bacc("TRN2", target_bir_lowering=False)
    f32 = mybir.dt.float32
    bf16 = mybir.dt.bfloat16
    fp8 = mybir.dt.float8e4
    DR = mybir.MatmulPerfMode.DoubleRow

    npair = len(pair_offs)
    CORR_BLK = 0
    PW_BLK = 24
    ID_BLK = 25
    n_blk = 26

    xp8_d = nc.dram_tensor("xp8", [C, HP * WP], fp8, kind="ExternalInput")
    xpb_d = nc.dram_tensor("xpb", [C, HP * WP], bf16, kind="ExternalInput")
    wt8_d = nc.dram_tensor("wt8", [C, npair * 2 * C], fp8,
                           kind="ExternalInput")
    wtb_d = nc.dram_tensor("wtb", [C, n_blk * C], bf16, kind="ExternalInput")
    xres_d = nc.dram_tensor("xres", [C, H * W], f32, kind="ExternalInput")
    ks_d = nc.dram_tensor("ks", [C, OFF_N], f32, kind="ExternalInput")
    beff_d = nc.dram_tensor("beff", [C, 1], f32, kind="ExternalInput")
    out_d = nc.dram_tensor("out", [C, H * W], f32, kind="ExternalOutput")

    with tile.TileContext(nc) as tc:
        with (
            tc.tile_pool(name="const", bufs=1) as const,
            tc.tile_pool(name="outp", bufs=3) as outp,
            tc.tile_pool(name="xrp", bufs=3) as xrp,
            tc.tile_pool(name="prodp", bufs=2) as prodp,
            tc.tile_pool(name="yp", bufs=2) as yp,
            tc.tile_pool(name="psum", bufs=3, space="PSUM") as psum_pool,
            tc.tile_pool(name="psumc", bufs=1, space="PSUM") as psumc_pool,
        ):
            xp8_sb = const.tile([C, HP * WP], fp8)
            xpb_sb = const.tile([C, HP * WP], bf16)
            wt8_sb = const.tile([C, npair * 2 * C], fp8)
            wtb_sb = const.tile([C, n_blk * C], bf16)
            ks_sb = const.tile([C, OFF_N], f32)
            beff_sb = const.tile([C, 1], f32)

            # SWDGE (nc.gpsimd) fans >=1MB transfers across the SDMA engines;
            # order: minimum needed for SB0 first.
            nc.gpsimd.dma_start(out=wt8_sb, in_=wt8_d[:, :])
            ROWS0 = SB_ROWS + 2 * PAD
            nc.gpsimd.dma_start(out=xp8_sb[:, :ROWS0 * WP],
                                in_=xp8_d[:, :ROWS0 * WP])
            nc.gpsimd.dma_start(out=wtb_sb, in_=wtb_d[:, :])
            ROWSY = Y_ROWS + PAD + 13
            nc.gpsimd.dma_start(out=xpb_sb[:, :ROWSY * WP],
                                in_=xpb_d[:, :ROWSY * WP])
            XP_CHUNK_ROWS = 62
            for r0_ in range(ROWS0, HP, XP_CHUNK_ROWS):
                r1_ = min(r0_ + XP_CHUNK_ROWS, HP)
                nc.gpsimd.dma_start(out=xp8_sb[:, r0_ * WP:r1_ * WP],
                                    in_=xp8_d[:, r0_ * WP:r1_ * WP])
            for r0_ in range(ROWSY, HP, XP_CHUNK_ROWS):
                r1_ = min(r0_ + XP_CHUNK_ROWS, HP)
                nc.gpsimd.dma_start(out=xpb_sb[:, r0_ * WP:r1_ * WP],
                                    in_=xpb_d[:, r0_ * WP:r1_ * WP])
            nc.sync.dma_start(out=beff_sb, in_=beff_d[:, :])
            nc.sync.dma_start(out=ks_sb, in_=ks_d[:, :])

            xpb3 = xpb_sb.rearrange("p (r w) -> p r w", w=WP)
            part8 = xp8_sb[:, :].ap.to_list()[0][0]
            xp8_base = xp8_sb[:, :].offset

            def wblk(i):
                return wtb_sb[:, i * C:(i + 1) * C]

            wt8r = wt8_sb.rearrange("p (n two m) -> p n two m", two=2, m=C)

            def pair_rhs(prow, i):
                (dyA, dxA), (dyB, dxB) = pair_offs[i]
                offA = (prow + dyA) * WP + PAD + dxA
                offB = (prow + dyB) * WP + PAD + dxB
                a = xp8_sb[:, :].copy()
                a.ap = bass_rust.VecI64Pair(
                    [(part8, C), (offB - offA, 2), (WP, SUB_ROWS), (1, W)])
                a.offset = xp8_base + offA
                return a

            corr_sb = const.tile([C, 8 * H], bf16)

            def emit_corrections():
                # needs the full bf16 image -> emitted after SB0's matmuls
                psum_c = psumc_pool.tile([C, 8 * H], f32, name="psum_c")
                for j, st in enumerate(strips):
                    for i, sh in enumerate(st["shifts"]):
                        if st["kind"] == "col":
                            rhs = xpb3[:, PAD + sh: PAD + sh + H,
                                       PAD + st["src"]: PAD + st["src"] + 1]
                        else:
                            rhs = xpb3[:, PAD + st["src"]: PAD + st["src"] + 1,
                                       PAD + sh: PAD + sh + W]
                        nc.tensor.matmul(psum_c[:, j * H:(j + 1) * H],
                                         wblk(CORR_BLK + 3 * j + i), rhs,
                                         start=(i == 0), stop=(i == 2))
                nc.scalar.copy(corr_sb, psum_c)

            # ---- main loop -------------------------------------------------
            n_sub = SB_ROWS // SUB_ROWS
            pair_y = None
            for s in range(N_SB):
                r0 = s * SB_ROWS

                # offloaded taps: products on 4x-mode tensor_scalar, summed
                # with 2x-mode tensor_tensor, full padded-width rows
                if s % 2 == 0:
                    y = yp.tile([C, Y_ROWS * WP], bf16, tag="y")
                    ybase = (PAD + r0) * WP
                    for t, (dy, dx) in enumerate(tap_offsets):
                        o0 = ybase + dy * WP + dx
                        xs = xpb_sb[:, o0: o0 + Y_ROWS * WP]
                        kcol = ks_sb[:, t:t + 1]
                        if t == 0:
                            nc.vector.tensor_scalar_mul(y, xs, kcol)
                        else:
                            p = prodp.tile([C, Y_ROWS * WP], bf16, tag="prod")
                            nc.vector.tensor_scalar_mul(p, xs, kcol)
                            nc.vector.tensor_add(y, y, p)
                    pair_y = y.rearrange("p (r w) -> p r w", w=WP)
                half = (s % 2) * SB_ROWS

                psum = psum_pool.tile([C, SB_ROWS * W], f32, tag="acc")
                for i in range(npair):
                    for u in range(n_sub):
                        prow = PAD + r0 + u * SUB_ROWS
                        nc.tensor.matmul(
                            psum[:, u * SUB_ROWS * W:(u + 1) * SUB_ROWS * W],
                            wt8r[:, i], pair_rhs(prow, i),
                            start=(i == 0), stop=False,
                            perf_mode=DR, tile_position=(0, 0))
                for u in range(n_sub):
                    yr = half + u * SUB_ROWS
                    nc.tensor.matmul(
                        psum[:, u * SUB_ROWS * W:(u + 1) * SUB_ROWS * W],
                        wblk(PW_BLK),
                        pair_y[:, yr:yr + SUB_ROWS, PAD:PAD + W],
                        start=False, stop=False)

                if s == 0:
                    emit_corrections()

                # fold strip corrections into PSUM: identity-weight matmuls
                psum3 = psum.rearrange("p (r w) -> p r w", w=W)
                strip_mms = []
                for j, st in enumerate(strips):
                    if st["kind"] == "col":
                        dst = psum3[:, 0:SB_ROWS,
                                    st["fixed_out"]:st["fixed_out"] + 1]
                        src = corr_sb[:, j * H + r0: j * H + r0 + SB_ROWS]
                        strip_mms.append((dst, src))
                    elif r0 <= st["fixed_out"] < r0 + SB_ROWS:
                        lr = st["fixed_out"] - r0
                        strip_mms.append((psum3[:, lr:lr + 1, :],
                                          corr_sb[:, j * H: j * H + W]))
                for i, (dst, src) in enumerate(strip_mms):
                    nc.tensor.matmul(dst, wblk(ID_BLK), src,
                                     start=False,
                                     stop=(i == len(strip_mms) - 1))

                xr = xrp.tile([C, SB_ROWS * W], f32, tag="xres")
                nc.sync.dma_start(out=xr,
                                  in_=xres_d[:, r0 * W:(r0 + SB_ROWS) * W])
                out_sb = outp.tile([C, SB_ROWS * W], f32)
                nc.scalar.activation(out_sb, psum,
                                     mybir.ActivationFunctionType.Identity,
                                     bias=beff_sb[:, 0:1],
                                     scale=1.0 / WSCALE)
                nc.vector.tensor_add(out_sb, out_sb, xr)
                nc.gpsimd.dma_start(out=out_d[:, r0 * W:(r0 + SB_ROWS) * W],
                                    in_=out_sb)
    nc.finalize()
    return nc


def _make_in_maps(inputs):
    import ml_dtypes
    f8 = ml_dtypes.float8_e4m3
    bf = ml_dtypes.bfloat16

    x = np.ascontiguousarray(inputs["x"], dtype=np.float32)
    wt8, wtb, ks, beff, pair_offs, tap_offsets, strips = _build_weights(inputs)
    if "nc" not in _CACHE:
        _CACHE["nc"] = _build_program(pair_offs, tap_offsets, strips)

    xpad8 = np.zeros((B, C, HP, WP), f8)
    xpad8[:, :, PAD:PAD + H, PAD:PAD + W] = x.astype(f8)
    xpadb = np.zeros((B, C, HP, WP), bf)
    xpadb[:, :, PAD:PAD + H, PAD:PAD + W] = x.astype(bf)
    beff_col = np.ascontiguousarray(beff.reshape(C, 1))
    return [
        {
            "xp8": np.ascontiguousarray(xpad8[b].reshape(C, HP * WP)),
            "xpb": np.ascontiguousarray(xpadb[b].reshape(C, HP * WP)),
            "wt8": wt8,
            "wtb": wtb,
            "xres": np.ascontiguousarray(x[b].reshape(C, H * W)),
            "ks": ks,
            "beff": beff_col,
        }
        for b in range(B)
    ]


def kernel(**inputs):
    in_maps = _make_in_maps(inputs)
    from concourse.bass_utils import run_bass_kernel_spmd
    res = run_bass_kernel_spmd(_CACHE["nc"], in_maps,
                               core_ids=list(range(N_CORES)))
    out = np.stack([res.results[b]["out"].reshape(C, H, W) for b in range(B)])
    return out.astype(np.float32)
